# revision 9
# baseline (speedup 1.0000x reference)
"""nn_ADConv kernel: data-parallel over batch N=8 across 8 NeuronCores.

Strategy (sharding_hint: "Data-parallel over batch N across M devices"):
  - Each core gets one image x_i [64, 56, 56]; weights are replicated.
  - BatchNorm runs in training mode (batch statistics over (N, H, W)), so the
    per-channel sum / sum-of-squares are computed locally and AllReduced
    across the 8 cores with lax.psum before normalizing.
  - The per-pixel basis contraction is computed in "t-space":
        bases_out[c,m,p] = sum_t y2[m,t,p] * cols2[c,t,p]
        cols2[c,t]       = depthwise-conv(x[c], bases_kernel[t])
    which avoids materializing the full 49-tap unfold (39MB/image) and maps
    the 7x7 window onto a grouped convolution.
  - Final 1x1 conv with coef, then gather (pmap stacks the shards).

Hardcoded problem shapes (must not read spec/reference at grade time):
  N=8, CIN=64, H=W=56, INTER=64, BS=108, M=6, T=18, KS=7, PAD=3, COUT=128.
"""

import jax
import jax.numpy as jnp
import numpy as np
from functools import partial

KS = 7
PAD = 3
M = 6
T = 18
BS = 108
INTER = 64
CIN, COUT = 64, 128
N, H, W = 8, 56, 56

_EPS = 1e-5


def _conv2d(x, w, pad):
    # x: [1, Cin, H, W], w: [Cout, Cin, kh, kw]
    return jax.lax.conv_general_dilated(
        x, w, (1, 1), [(pad, pad), (pad, pad)],
        dimension_numbers=("NCHW", "OIHW", "NCHW"),
    )


def _bn_tanh(z, g, b, axis_name):
    # z: [1, C, H, W]; training-mode BN over (N, H, W) via cross-core psum.
    # The conv bias that precedes BN cancels inside BN, so callers skip it;
    # g/b are the BN affine parameters.
    cnt = N * H * W
    s1 = jax.lax.psum(jnp.sum(z, axis=(0, 2, 3)), axis_name)          # [C]
    s2 = jax.lax.psum(jnp.sum(z * z, axis=(0, 2, 3)), axis_name)     # [C]
    mean = s1 / cnt
    var = s2 / cnt - mean * mean
    scale = g * jax.lax.rsqrt(var + _EPS)
    shift = b - mean * scale
    return jnp.tanh(z * scale[None, :, None, None] + shift[None, :, None, None])


def _per_core(x, conv1_w, conv1_b, bn1_g, bn1_b, conv2_w, conv2_b,
              bn2_g, bn2_b, coef, bases, axis_name="b"):
    # x: [1, CIN, H, W] (one image per core); math follows reference, with BN
    # batch statistics obtained via cross-core psum (computed in fp32).
    # Convolutions / contractions run with bf16 operands + fp32 accumulation
    # (PE accumulates in fp32); the 2e-2 rel-err budget easily covers it.
    del conv1_b, conv2_b  # constant channel bias cancels in training-mode BN
    bf = jnp.bfloat16
    f32 = jnp.float32

    def conv(a, w):
        return jax.lax.conv_general_dilated(
            a.astype(bf), w.astype(bf), (1, 1), [(1, 1), (1, 1)],
            dimension_numbers=("NCHW", "OIHW", "NCHW"),
            preferred_element_type=f32)

    y = _bn_tanh(conv(x, conv1_w), bn1_g, bn1_b, axis_name)
    y = _bn_tanh(conv(y, conv2_w), bn2_g, bn2_b, axis_name)         # [1,108,H,W]

    # t-space contraction: bases_out[c,m] = sum_t y2[m,t] * cols2[c,t] where
    # cols2[c,t] = depthwise-conv(x[c], bases[t]) — 18 terms instead of 49.
    kern = bases.reshape(T, KS, KS)
    dw = jnp.tile(kern[None], (CIN, 1, 1, 1)).reshape(CIN * T, 1, KS, KS)
    cols2 = jax.lax.conv_general_dilated(
        x.astype(bf), dw.astype(bf), (1, 1), [(PAD, PAD), (PAD, PAD)],
        dimension_numbers=("NCHW", "OIHW", "NCHW"),
        feature_group_count=CIN,
        preferred_element_type=f32,
    ).reshape(1, CIN, T, H, W).astype(bf)

    y2 = y.reshape(1, M, T, H, W).astype(bf)
    acc = jnp.zeros((1, CIN, M, H, W), f32)
    for t in range(T):
        acc = acc + (cols2[:, :, None, t] * y2[:, None, :, t]).astype(f32)
    bases_out = acc.reshape(1, CIN * M, H, W)                        # [1,384,H,W]
    out = jnp.einsum("bkhw,ok->bohw", bases_out.astype(bf),
                     coef.astype(bf), preferred_element_type=f32)
    return out.astype(f32)                                           # [1,128,H,W]


_CACHE = {}


def kernel(**inputs):
    xs = {k: np.asarray(v) for k, v in inputs.items()}
    x = xs["x"].astype(np.float32).reshape(N, 1, CIN, H, W)           # shard axis

    if "fn" not in _CACHE:
        _CACHE["fn"] = jax.pmap(partial(_per_core, axis_name="b"),
                                axis_name="b", devices=jax.devices()[:N])
    fn = _CACHE["fn"]

    import hashlib
    wkey = tuple((k, hashlib.md5(np.ascontiguousarray(xs[k])).hexdigest())
                 for k in sorted(xs) if k != "x")
    if _CACHE.get("wkey") != wkey or "w" not in _CACHE:
        rep = lambda a: jnp.asarray(
            np.broadcast_to(np.asarray(a, np.float32), (N,) + np.asarray(a).shape))
        _CACHE["w"] = [rep(xs[k]) for k in
                       ("conv1_w", "conv1_b", "bn1_g", "bn1_b",
                        "conv2_w", "conv2_b", "bn2_g", "bn2_b",
                        "coef", "bases")]
        _CACHE["wkey"] = wkey

    out = fn(x, *_CACHE["w"])
    return np.asarray(out).reshape(N, COUT, H, W).astype(np.float32)


# revision 10
# speedup vs baseline: 1.0002x; 1.0002x over previous
"""nn_ADConv kernel: data-parallel over batch N=8 across 8 NeuronCores.

Strategy (sharding_hint: "Data-parallel over batch N across M devices"):
  - Each core gets one image x_i [64, 56, 56]; weights are replicated.
  - BatchNorm runs in training mode (batch statistics over (N, H, W)), so the
    per-channel sum / sum-of-squares are computed locally and AllReduced
    across the 8 cores with lax.psum before normalizing.
  - The per-pixel basis contraction is computed in "t-space":
        bases_out[c,m,p] = sum_t y2[m,t,p] * cols2[c,t,p]
        cols2[c,t]       = depthwise-conv(x[c], bases_kernel[t])
    which avoids materializing the full 49-tap unfold (39MB/image) and maps
    the 7x7 window onto a grouped convolution.
  - Final 1x1 conv with coef, then gather (pmap stacks the shards).

Hardcoded problem shapes (must not read spec/reference at grade time):
  N=8, CIN=64, H=W=56, INTER=64, BS=108, M=6, T=18, KS=7, PAD=3, COUT=128.
"""

import jax
import jax.numpy as jnp
import numpy as np
from functools import partial

KS = 7
PAD = 3
M = 6
T = 18
BS = 108
INTER = 64
CIN, COUT = 64, 128
N, H, W = 8, 56, 56

_EPS = 1e-5


def _conv2d(x, w, pad):
    # x: [1, Cin, H, W], w: [Cout, Cin, kh, kw]
    return jax.lax.conv_general_dilated(
        x, w, (1, 1), [(pad, pad), (pad, pad)],
        dimension_numbers=("NCHW", "OIHW", "NCHW"),
    )


def _bn_tanh(z, g, b, axis_name):
    # z: [1, C, H, W]; training-mode BN over (N, H, W) via cross-core psum.
    # The conv bias that precedes BN cancels inside BN, so callers skip it;
    # g/b are the BN affine parameters.
    cnt = N * H * W
    # one fused AllReduce for [sum; sumsq] — halves the collective count
    # (each psum carries a ~10us hardware latency floor)
    loc = jnp.stack([jnp.sum(z, axis=(0, 2, 3)),
                     jnp.sum(z * z, axis=(0, 2, 3))])                # [2, C]
    s = jax.lax.psum(loc, axis_name)
    mean = s[0] / cnt
    var = s[1] / cnt - mean * mean
    scale = g * jax.lax.rsqrt(var + _EPS)
    shift = b - mean * scale
    return jnp.tanh(z * scale[None, :, None, None] + shift[None, :, None, None])


def _per_core(x, conv1_w, conv1_b, bn1_g, bn1_b, conv2_w, conv2_b,
              bn2_g, bn2_b, coef, bases, axis_name="b"):
    # x: [1, CIN, H, W] (one image per core); math follows reference, with BN
    # batch statistics obtained via cross-core psum (computed in fp32).
    # Convolutions / contractions run with bf16 operands + fp32 accumulation
    # (PE accumulates in fp32); the 2e-2 rel-err budget easily covers it.
    del conv1_b, conv2_b  # constant channel bias cancels in training-mode BN
    bf = jnp.bfloat16
    f32 = jnp.float32

    def conv(a, w):
        return jax.lax.conv_general_dilated(
            a.astype(bf), w.astype(bf), (1, 1), [(1, 1), (1, 1)],
            dimension_numbers=("NCHW", "OIHW", "NCHW"),
            preferred_element_type=f32)

    y = _bn_tanh(conv(x, conv1_w), bn1_g, bn1_b, axis_name)
    y = _bn_tanh(conv(y, conv2_w), bn2_g, bn2_b, axis_name)         # [1,108,H,W]

    # t-space contraction: bases_out[c,m] = sum_t y2[m,t] * cols2[c,t] where
    # cols2[c,t] = depthwise-conv(x[c], bases[t]) — 18 terms instead of 49.
    kern = bases.reshape(T, KS, KS)
    dw = jnp.tile(kern[None], (CIN, 1, 1, 1)).reshape(CIN * T, 1, KS, KS)
    cols2 = jax.lax.conv_general_dilated(
        x.astype(bf), dw.astype(bf), (1, 1), [(PAD, PAD), (PAD, PAD)],
        dimension_numbers=("NCHW", "OIHW", "NCHW"),
        feature_group_count=CIN,
        preferred_element_type=f32,
    ).reshape(1, CIN, T, H, W).astype(bf)

    y2 = y.reshape(1, M, T, H, W).astype(bf)
    acc = jnp.zeros((1, CIN, M, H, W), f32)
    for t in range(T):
        acc = acc + (cols2[:, :, None, t] * y2[:, None, :, t]).astype(f32)
    bases_out = acc.reshape(1, CIN * M, H, W)                        # [1,384,H,W]
    out = jnp.einsum("bkhw,ok->bohw", bases_out.astype(bf),
                     coef.astype(bf), preferred_element_type=f32)
    return out.astype(f32)                                           # [1,128,H,W]


_CACHE = {}


def kernel(**inputs):
    xs = {k: np.asarray(v) for k, v in inputs.items()}
    x = xs["x"].astype(np.float32).reshape(N, 1, CIN, H, W)           # shard axis

    if "fn" not in _CACHE:
        _CACHE["fn"] = jax.pmap(partial(_per_core, axis_name="b"),
                                axis_name="b", devices=jax.devices()[:N])
    fn = _CACHE["fn"]

    import hashlib
    wkey = tuple((k, hashlib.md5(np.ascontiguousarray(xs[k])).hexdigest())
                 for k in sorted(xs) if k != "x")
    if _CACHE.get("wkey") != wkey or "w" not in _CACHE:
        rep = lambda a: jnp.asarray(
            np.broadcast_to(np.asarray(a, np.float32), (N,) + np.asarray(a).shape))
        _CACHE["w"] = [rep(xs[k]) for k in
                       ("conv1_w", "conv1_b", "bn1_g", "bn1_b",
                        "conv2_w", "conv2_b", "bn2_g", "bn2_b",
                        "coef", "bases")]
        _CACHE["wkey"] = wkey

    out = fn(x, *_CACHE["w"])
    return np.asarray(out).reshape(N, COUT, H, W).astype(np.float32)
